# revision 7
# baseline (speedup 1.0000x reference)
"""Sparse BERT self-attention (DeBERTa-style one-pass mask) on 8 Trainium2
NeuronCores. Data-parallel over batch: core b handles batch element b.

v2: fp8(e4m3) DoubleRow projections at 2x PE rate + fp16 attention.

Design:
  - Host pre-transposes and quantizes: x^T and 64*W^T packed into fp8
    contraction-pair layout [3, 128, 2, cols] (DoubleRow: two 128-chunks of
    the contraction dim per matmul). 64x weight scale keeps W in e4m3's
    normal range; the kernel stays in "64-units" end to end (exp scale
    absorbs 1/4096, V's denominator ones-column holds 64).
  - Precision: fp8 projections give ~4% element error on Q/K/V, which
    averages out in the near-uniform cdd softmax (measured ~5e-3 rel err).
    The term self-attention path (q.q diagonal concentrates probs on self)
    does NOT average errors, so the 128 term rows of Q and V are
    re-projected in fp16 (tiny: 36+12 extra matmuls).
  - Scores are computed transposed (keys on partitions) only for the 192
    keys each query actually attends to; exp on ScalarE with scale fused.
  - Sig-ctx packing: per candidate pair j, a zero-initialized E tile
    [128 keys, 10, 128 q] holds exp scores in its diagonal 64x64 quadrants
    (2 strided DVE copies/head); the zero cross-quadrants make a single
    128-contraction ctx matmul per (head, s-tile) exact - half the sig-ctx
    matmul count vs 64-row pairs, at full partition width.
  - Head-group pipeline (2 groups of 6 heads) as before; input DMA spread
    across sync/scalar/vector queues in need-order; E-tile zeroing and V
    ones-columns on the otherwise-idle GpSimd engine.

Shapes (hardcoded per problem spec):
  B=8, S=1408, D=768, H=12, Dh=64, L=64 (signal), CDD=20, T=128 (terms),
  AF = CDD*L = 1280.

Math notes:
  - bk never enters: constant over keys -> cancels in softmax.
  - bq added as 64*bq during the Q psum drain (per-partition add).
  - bv added on host after normalization (sum_k p = 1 -> +bv once).
  - exp without max-subtraction: |scores| <= ~5, safe in fp32 psum.
"""

import sys

sys.path.insert(0, "/opt/trn_rl_repo")

import ml_dtypes
import numpy as np

import concourse.bass as bass
import concourse.mybir as mybir
import concourse.tile as tile
from concourse.bass_utils import run_bass_kernel_spmd

import json as _json

import concourse.bass2jax as _b2j
import concourse.bass_utils as _bu


def _dedup_ldweights(mod):
    """Drop PE Ldweights that reload the stationary already in the array.

    The bacc lowering emits one Ldweights per Matmult; with the projection
    loops ordered stationary-major, 2/3 of them are identical reloads
    (~100ns each on the PE). A redundant Ldweights carrying sem waits is
    replaced by a NoOp so the synchronization is preserved.
    """
    n_del = 0
    for fn in mod.get("functions", []):
        for bb in fn.get("blocks", []):
            out = []
            last_key = None
            for inst in bb.get("instructions", []):
                if inst.get("engine") != "PE":
                    out.append(inst)
                    continue
                op = inst.get("opcode")
                if op == "Ldweights":
                    key = _json.dumps(
                        {k: v for k, v in inst.items() if k not in ("name", "sync_info", "debug")},
                        sort_keys=True,
                    )
                    if key == last_key:
                        si = inst.get("sync_info")
                        if si and (si.get("on_wait") or si.get("on_update")):
                            out.append(
                                {
                                    "engine": "PE",
                                    "ins": [],
                                    "outs": [],
                                    "name": inst["name"] + "_n",
                                    "opcode": "NoOp",
                                    "sync_info": si,
                                }
                            )
                        n_del += 1
                        continue
                    last_key = key
                    out.append(inst)
                elif op == "Matmult" and inst.get("ldweights") is False and not inst.get("is_transpose"):
                    out.append(inst)
                else:
                    last_key = None
                    out.append(inst)
            bb["instructions"] = out
    return n_del


_orig_compile_bir_kernel = _bu.compile_bir_kernel


def _compile_bir_kernel_dedup(bir_json, tmpdir, neff_name="file.neff"):
    mod = _json.loads(bir_json)
    _dedup_ldweights(mod)
    return _orig_compile_bir_kernel(
        _json.dumps(mod).encode(), tmpdir, neff_name=neff_name
    )


_bu.compile_bir_kernel = _compile_bir_kernel_dedup
_b2j.compile_bir_kernel = _compile_bir_kernel_dedup

# ---------------------------------------------------------------- constants
B, S, D = 8, 1408, 768
H, Dh = 12, 64
L, CDD, T = 64, 20, 128
AF = CDD * L  # 1280
NDC = D // 128  # 6 chunks of the contraction dim
NDP = 3  # 3 DoubleRow pair-chunks (256 each)
NST = S // 128  # 11 s-tiles
NPAIR = 10  # candidate pairs
SC = 64.0  # weight scale (e4m3 range)
SS = (1.0 / 8.0) / (SC * SC)  # exp scale: 1/sqrt(Dh) / 64^2

F16 = mybir.dt.float16
FP8 = mybir.dt.float8e4
F32 = mybir.dt.float32
DR = mybir.MatmulPerfMode.DoubleRow

K_SCHUNKS = [(0, 512), (512, 1024), (1024, 1408)]
Q_SCHUNKS = [(0, 512), (512, 1024), (1024, 1280)]  # Q term cols via fp16 path
TERM_QCHUNKS = [(0, 512), (512, 1024), (1024, 1280)]  # cdd query chunks
V_OCHUNKS = [(0, 512), (512, 768)]
VT_OCHUNKS = [(0, 384), (384, 768)]


# --------------------------------------------- walrus sem-wait legalization
def _legalize_waits(nc, max_waits=1):
    """This container's walrus rejects more than one sem wait per
    instruction. Hoist excess waits onto NOPs inserted just before the
    instruction on the same engine (engine streams execute in block order,
    so the conjunction of waits is preserved)."""
    from concourse import mybir

    k = 0
    for fn in nc.m.functions:
        for bb in fn.blocks:
            new_list = []
            changed = False
            for inst in bb.instructions:
                si = inst.sync_info
                waits = list(si.on_wait) if si is not None else []
                if len(waits) > max_waits:
                    changed = True
                    for w in waits[:-max_waits]:
                        nop = mybir.InstNoOp(name=f"waitsplit_{k}", ins=[], outs=[])
                        k += 1
                        nop.engine = inst.engine
                        nop.sync_info = mybir.SyncInfo(on_wait=[w], on_update=[])
                        new_list.append(nop)
                    inst.sync_info = mybir.SyncInfo(
                        on_wait=waits[-max_waits:], on_update=list(si.on_update)
                    )
                new_list.append(inst)
            if changed:
                bb.instructions = new_list


def _patch_tile_teardown():
    """Drop the second all-engine barrier of the kernel-tail teardown."""
    import concourse.tile as tile_mod
    from concourse.vector_clock import ScopedClock

    def _patched(self, tick_clock, wait_clock):
        nc = self.nc
        drain_inst = nc.sync.drain()
        wait_clock.add_sem_waits(
            drain_inst.ins, ScopedClock({None: tick_clock.global_clock})
        )
        nc.all_engine_barrier()
        assert self.sems is not None
        popped = nc._tile_sem_poison_stack.pop()
        assert popped is self._sem_poison
        nc.clear_and_free_semaphores(list(self.sems.allocated().values()))

    tile_mod.TileContext._drain_and_barrier = _patched


_patch_tile_teardown()


# ------------------------------------------------------------ bass program
def _build_program():
    nc = bass.Bass()
    AF_ = mybir.ActivationFunctionType

    xp_d = nc.dram_tensor("xp", [NDP, 128, 2, S], FP8, kind="ExternalInput")
    wqp_d = nc.dram_tensor("wqp", [NDP, 128, 2, D], FP8, kind="ExternalInput")
    wkp_d = nc.dram_tensor("wkp", [NDP, 128, 2, D], FP8, kind="ExternalInput")
    wvp_d = nc.dram_tensor("wvp", [NDP, 128, 2, D], FP8, kind="ExternalInput")
    xt16_d = nc.dram_tensor("xt16", [128, NDC, T], F16, kind="ExternalInput")
    wq16_d = nc.dram_tensor("wq16", [128, NDC, D], F16, kind="ExternalInput")
    wv16_d = nc.dram_tensor("wv16", [128, NDC, D], F16, kind="ExternalInput")
    bq_d = nc.dram_tensor("bq", [128, NDC], F32, kind="ExternalInput")
    out_d = nc.dram_tensor("out", [S, D], F32, kind="ExternalOutput")

    with tile.TileContext(nc) as tc:
        with (
            tc.tile_pool(name="persist", bufs=1) as pp,
            tc.tile_pool(name="exps", bufs=2) as ep,
            tc.tile_pool(name="misc", bufs=4) as mp,
        ):
            # ---------------- input DMA (need-order, 3 queues)
            bq_all = pp.tile([128, NDC], F32, name="bq_all", tag="bq_all")
            xpt, wt = [], {"q": [], "k": [], "v": []}
            for j in range(NDP):
                t = pp.tile([128, 2, S], FP8, name=f"xp{j}", tag=f"xp{j}")
                nc.sync.dma_start(out=t, in_=xp_d[j])
                xpt.append(t)
                w = pp.tile([128, 2, D], FP8, name=f"wkp{j}", tag=f"wkp{j}")
                nc.sync.dma_start(out=w, in_=wkp_d[j])
                wt["k"].append(w)
                w = pp.tile([128, 2, D], FP8, name=f"wqp{j}", tag=f"wqp{j}")
                nc.scalar.dma_start(out=w, in_=wqp_d[j])
                wt["q"].append(w)
            nc.scalar.dma_start(out=bq_all, in_=bq_d[:, :])
            for j in range(NDP):
                w = pp.tile([128, 2, D], FP8, name=f"wvp{j}", tag=f"wvp{j}")
                nc.sync.dma_start(out=w, in_=wvp_d[j])
                wt["v"].append(w)
            bqt = [bq_all[:, j : j + 1] for j in range(NDC)]

            QT = [pp.tile([128, AF], F16, name=f"qT{j}", tag=f"qT{j}") for j in range(NDC)]
            KT = [pp.tile([128, S], F16, name=f"kT{j}", tag=f"kT{j}") for j in range(NDC)]
            # V tiles: [128, H, Dh+1]; column Dh holds 64.0 (denominator,
            # 64-units).
            V = [pp.tile([128, H, Dh + 1], F16, name=f"v{st}", tag=f"v{st}") for st in range(NST)]
            QTerm = [pp.tile([128, T], F16, name=f"qt{j}", tag=f"qt{j}") for j in range(NDC)]
            # E tiles: [128 keys, pair, 128 q]; cross-quadrants stay ZERO so
            # the packed 128-contraction sig-ctx matmul is exact.
            E = [pp.tile([128, NPAIR, 128], F16, name=f"E{h}", tag=f"E{h}") for h in range(H)]
            # gpsimd stream order: warm-up source first (gates the PE clock
            # ramp), then the cheap V ones-columns, then the E zero-fills
            wsrc = pp.tile([128, 512], F16, name="warm_src", tag="warm_src")
            nc.gpsimd.memset(wsrc, 1.0)
            for st in range(NST):
                nc.gpsimd.memset(V[st][:, :, Dh : Dh + 1], SC)
            for h in range(H):
                nc.gpsimd.memset(E[h], 0.0)

            # ---------------- projections (fp8 DoubleRow, stationary reuse)
            with tc.tile_pool(name="pproj", bufs=7, space=bass.MemorySpace.PSUM) as pj:
                  # PE clock warm-up (~3.4us of activity to reach 2.4GHz)
                  wps = pj.tile([128, 512], F32, name="warm_ps", tag="proj")
                  for _ in range(10):
                      nc.tensor.matmul(
                          wps, lhsT=wsrc[:, 0:128], rhs=wsrc, start=True, stop=True
                      )
                  nc.scalar.activation(
                      out=wsrc[:, 0:1], in_=wps[:, 0:1], func=AF_.Copy
                  )
                  for oc in range(NDC):
                      # Q: 3 psum banks accumulate over dcp; one LDWEIGHTS
                      # per (oc, dcp) serves all 3 s-chunks
                      pqs = [pj.tile([128, 512], F32, name=f"pq{si}", tag="proj") for si in range(3)]
                      for dcp in range(NDP):
                          lhs = wt["q"][dcp][:, :, oc * 128 : (oc + 1) * 128]
                          for si, (s0, s1) in enumerate(Q_SCHUNKS):
                              nc.tensor.matmul(
                                  pqs[si][:, : s1 - s0],
                                  lhsT=lhs,
                                  rhs=xpt[dcp][:, :, s0:s1],
                                  start=(dcp == 0),
                                  stop=(dcp == NDP - 1),
                                  perf_mode=DR,
                              )
                      # Q^T = psum + 64*bq (per-partition), cast to fp16
                      nc.vector.tensor_scalar_add(
                          out=QT[oc][:, 0:512], in0=pqs[0], scalar1=bqt[oc]
                      )
                      nc.vector.tensor_scalar_add(
                          out=QT[oc][:, 512:1024], in0=pqs[1], scalar1=bqt[oc]
                      )
                      nc.scalar.activation(
                          out=QT[oc][:, 1024:1280],
                          in_=pqs[2][:, :256],
                          func=AF_.Identity,
                          bias=bqt[oc],
                      )
                      pks = [pj.tile([128, 512], F32, name=f"pk{si}", tag="proj") for si in range(3)]
                      for dcp in range(NDP):
                          lhs = wt["k"][dcp][:, :, oc * 128 : (oc + 1) * 128]
                          for si, (s0, s1) in enumerate(K_SCHUNKS):
                              nc.tensor.matmul(
                                  pks[si][:, : s1 - s0],
                                  lhsT=lhs,
                                  rhs=xpt[dcp][:, :, s0:s1],
                                  start=(dcp == 0),
                                  stop=(dcp == NDP - 1),
                                  perf_mode=DR,
                              )
                      nc.scalar.activation(
                          out=KT[oc][:, 0:512], in_=pks[0], func=AF_.Copy
                      )
                      nc.vector.tensor_copy(
                          out=KT[oc][:, 512:1024], in_=pks[1]
                      )
                      nc.scalar.activation(
                          out=KT[oc][:, 1024:1408], in_=pks[2][:, :384], func=AF_.Copy
                      )
                      if oc == 2:
                          # late bulky fp16 inputs (needed ~25us in)
                          xt16 = pp.tile([128, NDC, T], F16, name="xt16", tag="xt16")
                          nc.scalar.dma_start(out=xt16, in_=xt16_d[:, :, :])
                          wq16 = pp.tile([128, NDC, D], F16, name="wq16", tag="wq16")
                          nc.scalar.dma_start(out=wq16, in_=wq16_d[:, :, :])
                          wv16 = pp.tile([128, NDC, D], F16, name="wv16", tag="wv16")
                          nc.scalar.dma_start(out=wv16, in_=wv16_d[:, :, :])
                  # V projection for s-tiles 0..9 (fp8); tile 10 via fp16 path
                  for st in range(NST - 1):
                      pvs = [pj.tile([128, 512], F32, name=f"pv{vi}", tag="proj") for vi in range(2)]
                      for dcp in range(NDP):
                          lhs = xpt[dcp][:, :, st * 128 : (st + 1) * 128]
                          for vi, (o0, o1) in enumerate(V_OCHUNKS):
                              nc.tensor.matmul(
                                  pvs[vi][:, : o1 - o0],
                                  lhsT=lhs,
                                  rhs=wt["v"][dcp][:, :, o0:o1],
                                  start=(dcp == 0),
                                  stop=(dcp == NDP - 1),
                                  perf_mode=DR,
                              )
                      nc.scalar.activation(
                          out=V[st][:, 0:8, 0:Dh],
                          in_=pvs[0].rearrange("p (h d) -> p h d", d=Dh),
                          func=AF_.Copy,
                      )
                      nc.vector.tensor_copy(
                          out=V[st][:, 8:12, 0:Dh],
                          in_=pvs[1][:, :256].rearrange("p (h d) -> p h d", d=Dh),
                      )
                  # fp16 term re-projection: Q[AF:] (pst path) and V[AF:]
                  for oc in range(NDC):
                      pq = pj.tile([128, 512], F32, name="pq0", tag="proj")
                      for dc in range(NDC):
                          nc.tensor.matmul(
                              pq[:, :T],
                              lhsT=wq16[:, dc, oc * 128 : (oc + 1) * 128],
                              rhs=xt16[:, dc, :],
                              start=(dc == 0),
                              stop=(dc == NDC - 1),
                          )
                      nc.vector.tensor_scalar_add(
                          out=QTerm[oc], in0=pq[:, :T], scalar1=bqt[oc]
                      )
                  pvt = [pj.tile([128, 512], F32, name=f"pv{vi}", tag="proj") for vi in range(2)]
                  for dc in range(NDC):
                      for vi, (o0, o1) in enumerate(VT_OCHUNKS):
                          nc.tensor.matmul(
                              pvt[vi][:, :384],
                              lhsT=xt16[:, dc, :],
                              rhs=wv16[:, dc, o0:o1],
                              start=(dc == 0),
                              stop=(dc == NDC - 1),
                          )
                  nc.scalar.activation(
                      out=V[NST - 1][:, 0:6, 0:Dh],
                      in_=pvt[0][:, :384].rearrange("p (h d) -> p h d", d=Dh),
                      func=AF_.Copy,
                  )
                  nc.vector.tensor_copy(
                      out=V[NST - 1][:, 6:12, 0:Dh],
                      in_=pvt[1][:, :384].rearrange("p (h d) -> p h d", d=Dh),
                  )

            # ------- head-group pipeline: scores+exp for 6 heads, then ctx
            with (
                tc.tile_pool(name="pst", bufs=2, space=bass.MemorySpace.PSUM) as pst,
                tc.tile_pool(name="psg", bufs=1, space=bass.MemorySpace.PSUM) as psg,
                tc.tile_pool(name="psm", bufs=1, space=bass.MemorySpace.PSUM) as psm,
                tc.tile_pool(name="pctx", bufs=3, space=bass.MemorySpace.PSUM) as pctx,
            ):
                for hg in range(2):
                    ET, EP = {}, {}
                    for hpair in range(3):
                        h0 = hg * 6 + hpair * 2
                        j = h0 // 2
                        qa, ka = QT[j][0:Dh, :], KT[j][0:Dh, :]
                        qb, kb = QT[j][Dh:128, :], KT[j][Dh:128, :]

                        # term scores for both heads of the pair
                        for h, qh, kh in ((h0, qa, ka), (h0 + 1, qb, kb)):
                            et = ep.tile([128, AF], F16, name=f"et{h % 6}", tag=f"et{h % 6}")
                            for s0, s1 in TERM_QCHUNKS:
                                w = s1 - s0
                                stp = pst.tile([128, 512], F32, name="stp", tag="st")
                                nc.tensor.matmul(
                                    stp[:, :w],
                                    lhsT=kh[:, AF:S],
                                    rhs=qh[:, s0:s1],
                                    start=True,
                                    stop=True,
                                )
                                nc.scalar.activation(
                                    out=et[:, s0:s1],
                                    in_=stp[:, :w],
                                    func=AF_.Exp,
                                    scale=SS,
                                )
                            ET[h] = et

                        # sig scores: interleave the two heads with opposite
                        # candidate parity -> disjoint (row, col) quadrants
                        sg = {}
                        for h in (h0, h0 + 1):
                            sg[h] = (
                                psg.tile([128, 512], F32, name=f"sga{h%2}", tag=f"sga{h%2}"),
                                psm.tile([128, 128], F32, name=f"sgb{h%2}", tag="small"),
                            )
                        for c0 in range(CDD):
                            for h, qh, kh, c in (
                                (h0, qa, ka, c0),
                                (h0 + 1, qb, kb, c0 ^ 1),
                            ):
                                row = (c % 2) * Dh
                                sga, sgb = sg[h]
                                if c < 16:
                                    dst = sga[
                                        row : row + Dh,
                                        (c // 2) * 64 : (c // 2) * 64 + 64,
                                    ]
                                else:
                                    cb = (c // 2 - 8) * 64
                                    dst = sgb[row : row + Dh, cb : cb + 64]
                                nc.tensor.matmul(
                                    dst,
                                    lhsT=kh[:, c * L : (c + 1) * L],
                                    rhs=qh[:, c * L : (c + 1) * L],
                                    start=True,
                                    stop=True,
                                )
                        for h, qh, kh in ((h0, qa, ka), (h0 + 1, qb, kb)):
                            sga, sgb = sg[h]
                            eg = mp.tile([128, 640], F16, name="eg", tag="eg", bufs=4)
                            nc.scalar.activation(
                                out=eg[:, 0:512], in_=sga, func=AF_.Exp, scale=SS
                            )
                            nc.scalar.activation(
                                out=eg[:, 512:640], in_=sgb, func=AF_.Exp, scale=SS
                            )
                            # scatter diag quadrants into the zeroed E tile:
                            # rows 0:64 = even candidate (q cols 0:64),
                            # rows 64:128 = odd candidate (q cols 64:128)
                            nc.vector.tensor_copy(
                                out=E[h][0:Dh, :, 0:Dh],
                                in_=eg[0:Dh, :].rearrange("p (j c) -> p j c", c=64),
                            )
                            nc.vector.tensor_copy(
                                out=E[h][Dh:128, :, Dh:128],
                                in_=eg[Dh:128, :].rearrange("p (j c) -> p j c", c=64),
                            )
                            # term-term (pst) scores from fp16 QTerm
                            qt = QTerm[j][0:Dh, :] if h == h0 else QTerm[j][Dh:128, :]
                            spp = psm.tile([128, 128], F32, name="spp", tag="small")
                            nc.tensor.matmul(
                                spp, lhsT=qt, rhs=qt, start=True, stop=True
                            )
                            epp = ep.tile([128, T], F16, name=f"ep{h % 6}", tag=f"ep{h % 6}")
                            nc.scalar.activation(
                                out=epp, in_=spp, func=AF_.Exp, scale=SS
                            )
                            EP[h] = epp

                    for t in range(NST):
                        cps = pctx.tile([128, 6, Dh + 1], F32, name="cps", tag="ctx")
                        for hi in range(6):
                            h = hg * 6 + hi
                            nc.tensor.matmul(
                                cps[:, hi, :],
                                lhsT=ET[h][:, t * 128 : (t + 1) * 128]
                                if t < 10
                                else EP[h],
                                rhs=V[NST - 1][:, h, :],
                                start=(hi == 0),
                                stop=(t == 10 and hi == 5),
                            )
                        if t < 10:
                            for hi in range(6):
                                h = hg * 6 + hi
                                nc.tensor.matmul(
                                    cps[:, hi, :],
                                    lhsT=E[h][:, t, :],
                                    rhs=V[t][:, h, :],
                                    start=False,
                                    stop=(hi == 5),
                                )
                        rc = mp.tile([128, 6], F32, name="rc", tag="rc")
                        nc.vector.reciprocal(out=rc, in_=cps[:, :, Dh : Dh + 1])
                        ot = mp.tile([128, 6, Dh], F32, name="ot", tag="ot", bufs=6)
                        nc.vector.tensor_mul(
                            out=ot,
                            in0=cps[:, :, 0:Dh],
                            in1=rc.to_broadcast([128, 6, Dh]),
                        )
                        nc.sync.dma_start(
                            out=out_d[
                                t * 128 : (t + 1) * 128, hg * 384 : (hg + 1) * 384
                            ],
                            in_=ot,
                        )

    _legalize_waits(nc)
    return nc


_NC = None


def _get_nc():
    global _NC
    if _NC is None:
        _NC = _build_program()
    return _NC


# -------------------------------------------------------------- host wrapper
def _pair8(mT, cols):
    """[768, cols] f32 -> [3, 128, 2, cols] e4m3 DoubleRow pair layout."""
    E4 = ml_dtypes.float8_e4m3
    return np.ascontiguousarray(
        mT.reshape(NDP, 2, 128, cols).transpose(0, 2, 1, 3)
    ).astype(E4)


def _prep_inputs(hidden_states, Wq, bq, Wk, Wv):
    hs = np.asarray(hidden_states, dtype=np.float32)
    wqT = np.ascontiguousarray(np.asarray(Wq, np.float32).T) * SC
    wkT = np.ascontiguousarray(np.asarray(Wk, np.float32).T) * SC
    wvT = np.ascontiguousarray(np.asarray(Wv, np.float32).T) * SC
    bq64 = np.ascontiguousarray(
        (SC * np.asarray(bq, np.float32)).reshape(NDC, 128).T
    )

    wqp = _pair8(wqT, D)
    wkp = _pair8(wkT, D)
    wvp = _pair8(wvT, D)
    # [128, NDC, D] fp16 chunk layouts for the term re-projection
    wq16 = np.ascontiguousarray(
        wqT.reshape(NDC, 128, D).transpose(1, 0, 2)
    ).astype(np.float16)
    wv16 = np.ascontiguousarray(
        wvT.reshape(NDC, 128, D).transpose(1, 0, 2)
    ).astype(np.float16)

    in_maps = []
    for b in range(B):
        xT = np.ascontiguousarray(hs[b].T)  # [D, S] f32
        xp = _pair8(xT, S)
        xt16 = np.ascontiguousarray(
            xT[:, AF:].reshape(NDC, 128, T).transpose(1, 0, 2)
        ).astype(np.float16)
        in_maps.append(
            {
                "xp": xp,
                "xt16": xt16,
                "wqp": wqp,
                "wkp": wkp,
                "wvp": wvp,
                "wq16": wq16,
                "wv16": wv16,
                "bq": bq64,
            }
        )
    return in_maps


def _enable_tracing():
    """This image lacks ``antenv.axon_hooks``; recreate the NTFF profile hook
    from the boot package's ctypes impl, and defang the artifact upload."""
    import types

    import antenv

    if "antenv.axon_hooks" not in sys.modules:
        from trn_agent_boot.trn_boot import _ntff_profile_via_ctypes

        hook = _ntff_profile_via_ctypes("/opt/axon/libaxon_pjrt.so")
        mod = types.ModuleType("antenv.axon_hooks")
        mod.get_axon_ntff_profile_hook = lambda: hook
        mod.set_axon_ntff_profile_hook = lambda h: None
        sys.modules["antenv.axon_hooks"] = mod
        antenv.axon_hooks = mod
    import concourse.bass_utils as bu

    bu.upload_artifacts = lambda tmpdir: tmpdir


def run(inputs, trace=False, tmpdir=None):
    """Returns (output [B,S,D] f32, BassKernelResults)."""
    if trace:
        _enable_tracing()
    assert int(inputs["num_heads"]) == H
    assert int(inputs["signal_length"]) == L
    assert int(inputs["cdd_size"]) == CDD
    assert int(inputs["term_num"]) == T
    nc = _get_nc()
    in_maps = _prep_inputs(
        inputs["hidden_states"],
        inputs["Wq"],
        inputs["bq"],
        inputs["Wk"],
        inputs["Wv"],
    )
    res = run_bass_kernel_spmd(
        nc, in_maps, list(range(B)), trace=trace, tmpdir=tmpdir
    )
    out = np.stack([res.results[c]["out"] for c in range(B)]).astype(np.float32)
    out += np.asarray(inputs["bv"], dtype=np.float32)[None, None, :]
    return out, res


def kernel(**inputs) -> np.ndarray:
    out, _ = run(inputs, trace=False)
    return out
